# revision 1
# baseline (speedup 1.0000x reference)
"""Bass/Trainium2 kernel for nn_Attention_19481971654841.

Full attention block: q/k/v proj + per-head RMSNorm(q,k) + RoPE + causal GQA
SDPA + o_proj.  B=2, L=2048, D=1024, H=16, KVH=8, HD=128.

Sharding (8 NeuronCores): data-parallel over batch (2 groups of 4 cores) x
4-way tensor-parallel over heads inside each group.  Core c handles batch
c//4 and q-heads [4g:4g+4), kv-heads [2g:2g+2) with g = c%4.  Each core
produces a partial [L, D] o_proj contribution; host sums the 4 partials per
batch.

Per-core dataflow (all matmuls bf16, fp32 PSUM accumulate):
  - projections produce Q^T/K^T head-major [HD=128 part, T] (weights
    stationary on PE); V token-major [T, HD] via X-stationary matmuls.
  - RMSNorm in head-major: sum-of-squares via all-ones matmul (partition
    reduce, broadcast to 128 rows), ACT Sqrt, DVE fast reciprocal, fused
    (q * w) * r via scalar_tensor_tensor.
  - RoPE with host-prepared phase tables (args range-reduced to (-pi, pi]);
    sin table carries the rotate-half sign per partition so rope is
    4 full-width DVE ops per plane.
  - attention in S^T orientation: scores[kv, q] = K_hm^T . Q_hm, causal mask
    folded in as an identity-matmul bias add, ACT Exp -> P^T bf16, softmax
    denominator via all-ones matmul (broadcast rows), PV accumulates
    attn^T[hd, q] directly (no transposes anywhere).
  - o_proj: attn^T slices are the stationary lhsT; partial out written fp32
    from PSUM straight to DRAM.
"""

import math
from contextlib import ExitStack

import numpy as np
import ml_dtypes

import concourse.bass as bass
import concourse.mybir as mybir
import concourse.tile as tile

F32 = mybir.dt.float32
BF16 = mybir.dt.bfloat16
AF = mybir.ActivationFunctionType
ALU = mybir.AluOpType

# problem constants (per spec; hardcoded — kernel.py must be self-contained)
B, L, D = 2, 2048, 1024
H, KVH, HD = 16, 8, 128
EPS = 1e-6
ROPE_BASE = 1000000.0

# per-core constants
NCORES = 8
TPG = 4                 # tensor-parallel group size (cores per batch)
T = L                   # tokens per core (one batch element)
NQ = H // TPG           # 4 q heads per core
NKV = KVH // TPG        # 2 kv heads per core
DCH = D // 128          # 8 input-dim chunks
QT = 1024               # q tile width (PSUM tile [128, 1024] f32 = 2 banks)
NQT = T // QT           # 2 q tiles
NCH = T // 128          # 16 token chunks
MASK_NEG = -30000.0


def _chunks512(c0, end):
    """bank-aligned matmul col chunks covering [c0, end)"""
    out = []
    n0 = c0
    while n0 < end:
        ne = min(end, (n0 // 512 + 1) * 512)
        out.append((n0, ne))
        n0 = ne
    return out


def build_nc(parts="full"):
    nc = bass.Bass()

    xt_d = nc.dram_tensor("xt", [DCH, 128, T], BF16, kind="ExternalInput")
    wq_d = nc.dram_tensor("wq", [DCH, 128, NQ * HD], BF16, kind="ExternalInput")
    wk_d = nc.dram_tensor("wk", [DCH, 128, NKV * HD], BF16, kind="ExternalInput")
    wv_d = nc.dram_tensor("wv", [DCH, 128, NKV * HD], BF16, kind="ExternalInput")
    wo_d = nc.dram_tensor("wo", [NQ, 128, D], BF16, kind="ExternalInput")
    fqc_d = nc.dram_tensor("fqc", [128, T], F32, kind="ExternalInput")
    fqs_d = nc.dram_tensor("fqs", [128, T], F32, kind="ExternalInput")
    qnw_d = nc.dram_tensor("qnw", [128, 1], F32, kind="ExternalInput")
    knw_d = nc.dram_tensor("knw", [128, 1], F32, kind="ExternalInput")
    ones_d = nc.dram_tensor("ones", [128, 128], BF16, kind="ExternalInput")
    ident_d = nc.dram_tensor("ident", [128, 128], BF16, kind="ExternalInput")
    tri_d = nc.dram_tensor("tri", [128, 128], BF16, kind="ExternalInput")
    perm_d = nc.dram_tensor("perm", [128, 128], BF16, kind="ExternalInput")
    out_d = nc.dram_tensor("out", [NCH, 128, D], F32, kind="ExternalOutput")

    with tile.TileContext(nc) as tc, ExitStack() as ctx:
        sing = ctx.enter_context(tc.tile_pool(name="sing", bufs=1))
        trans = ctx.enter_context(tc.tile_pool(name="trans", bufs=2))
        pts = ctx.enter_context(tc.tile_pool(name="pts", bufs=4))
        psum = ctx.enter_context(tc.tile_pool(name="psum", bufs=1, space="PSUM"))

        # ---- persistent loads (wk/wq first, xt chunked: the first proj
        # matmuls only need wk + xt chunk 0, so PE starts ~3us in) ---------
        wq = sing.tile([128, DCH, NQ * HD], BF16, tag="wq")
        wk = sing.tile([128, DCH, NKV * HD], BF16, tag="wk")
        wv = sing.tile([128, DCH, NKV * HD], BF16, tag="wv")
        nc.sync.dma_start(out=wk, in_=wk_d.rearrange("d p f -> p d f"))
        xt = sing.tile([128, DCH, T], BF16, tag="xt")
        for d in range(DCH):
            nc.sync.dma_start(out=xt[:, d, :], in_=xt_d[d])
        nc.sync.dma_start(out=wq, in_=wq_d.rearrange("d p f -> p d f"))
        nc.sync.dma_start(out=wv, in_=wv_d.rearrange("d p f -> p d f"))
        wo = sing.tile([128, NQ, D], BF16, tag="wo")
        nc.sync.dma_start(out=wo, in_=wo_d.rearrange("h p f -> p h f"))
        fqc = sing.tile([128, T], F32, tag="fqc")
        fqs = sing.tile([128, T], F32, tag="fqs")
        nc.sync.dma_start(out=fqc, in_=fqc_d[:, :])
        nc.sync.dma_start(out=fqs, in_=fqs_d[:, :])
        qnw = sing.tile([128, 1], F32, tag="qnw")
        knw = sing.tile([128, 1], F32, tag="knw")
        nc.sync.dma_start(out=qnw, in_=qnw_d[:, :])
        nc.sync.dma_start(out=knw, in_=knw_d[:, :])
        ones = sing.tile([128, 128], BF16, tag="ones")
        ident = sing.tile([128, 128], BF16, tag="ident")
        tri = sing.tile([128, 128], BF16, tag="tri")
        perm = sing.tile([128, 128], BF16, tag="perm")
        nc.sync.dma_start(out=ones, in_=ones_d[:, :])
        nc.sync.dma_start(out=ident, in_=ident_d[:, :])
        nc.sync.dma_start(out=tri, in_=tri_d[:, :])
        nc.sync.dma_start(out=perm, in_=perm_d[:, :])
        epsb = sing.tile([128, 1], F32, tag="epsb")
        nc.vector.memset(epsb, EPS)

        # ---- rope tables: C = cos(pos*invf), Ssig = +-sin (sign folded) --
        ctab = sing.tile([128, T], BF16, tag="ctab")
        stab = sing.tile([128, T], BF16, tag="stab")
        nc.scalar.activation(out=ctab, in_=fqc, func=AF.Sin)
        nc.scalar.activation(out=stab, in_=fqs, func=AF.Sin)

        # ---- persistent plane outputs -----------------------------------
        khm = [sing.tile([128, T], BF16, tag=f"khm{i}", name=f"khm{i}") for i in range(NKV)]
        qhm = [sing.tile([128, T], BF16, tag=f"qhm{i}", name=f"qhm{i}") for i in range(NQ)]
        vsb = sing.tile([128, NKV, T], BF16, tag="vsb")
        attn = [sing.tile([128, T], BF16, tag=f"attn{i}", name=f"attn{i}") for i in range(NQ)]

        # ---- norm + rope pipeline, split into stages so plane units can
        # be software-pipelined against each other and against attention.
        # Unit = (plane, half).  Stage F (front): projection matmuls + early
        # PSUM drain to bf16 + square.  Stage B (back): partition-reduce
        # sum-of-squares (ones matmul), sqrt, reciprocal, fused norm scale,
        # rotate-half matmul, rope multiplies.
        def unit_front(w_ap, wmat, fslice, half):
            qp = psum.tile([128, QT], F32, tag="pp", bufs=2, name="qp")
            for d in range(DCH):
                for n0 in range(0, QT, 512):
                    nc.tensor.matmul(
                        qp[:, n0:n0 + 512],
                        lhsT=wmat[:, d, fslice],
                        rhs=xt[:, d, half * QT + n0: half * QT + n0 + 512],
                        start=(d == 0), stop=(d == DCH - 1),
                    )
            qc = trans.tile([128, QT], BF16, tag="qc", bufs=3, name="qc")
            nc.vector.tensor_copy(qc, qp)      # early PSUM drain
            sq = trans.tile([128, QT], BF16, tag="sq", bufs=2, name="sq")
            nc.vector.tensor_mul(sq, qc, qc)
            return (w_ap, half, qc, sq)

        def unit_back(plane_out, st):
            w_ap, half, qc, sq = st
            ssq = psum.tile([128, QT], F32, tag="pacc", bufs=1, name="ssq")
            for n0 in range(0, QT, 512):
                nc.tensor.matmul(ssq[:, n0:n0 + 512], lhsT=ones,
                                 rhs=sq[:, n0:n0 + 512], start=True, stop=True)
            ss = trans.tile([128, QT], F32, tag="ss", name="ss")
            nc.scalar.activation(out=ss, in_=ssq, func=AF.Sqrt,
                                 scale=1.0 / HD, bias=epsb)
            rr = trans.tile([128, QT], F32, tag="rr", name="rr")
            nc.vector.reciprocal(out=rr, in_=ss)
            qn = trans.tile([128, QT], BF16, tag="qn", name="qn")
            nc.vector.scalar_tensor_tensor(
                out=qn, in0=qc, scalar=w_ap, in1=rr,
                op0=ALU.mult, op1=ALU.mult)
            # rope: rotate-half via permutation matmul on PE, sign is
            # folded into the sin table (stab rows 0:64 hold -sin)
            cs = slice(half * QT, half * QT + QT)
            mc = trans.tile([128, QT], BF16, tag="mc", name="mc")
            nc.vector.tensor_mul(mc, qn, ctab[:, cs])
            rot = psum.tile([128, QT], F32, tag="pd", bufs=1, name="rot")
            for n0 in range(0, QT, 512):
                nc.tensor.matmul(rot[:, n0:n0 + 512], lhsT=perm,
                                 rhs=qn[:, n0:n0 + 512],
                                 start=True, stop=True)
            msw = trans.tile([128, QT], BF16, tag="msw", name="msw")
            nc.vector.tensor_mul(msw, rot, stab[:, cs])
            # final rope add on the (otherwise idle) gpsimd engine
            nc.gpsimd.tensor_add(plane_out[:, cs], mc, msw)

        # kv plane units, depth-2 pipelined
        kunits = [(khm[i], knw, wk, slice(i * HD, (i + 1) * HD), half)
                  for i in range(NKV) for half in range(NQT)]
        kunits += [(qhm[0], qnw, wq, slice(0, HD), half)
                   for half in range(NQT)]
        pend = []
        for plane, w_ap, wmat, fsl, half in kunits:
            st = unit_front(w_ap, wmat, fsl, half)
            pend.append((plane, st))
            if len(pend) > 1:
                unit_back(*pend.pop(0))
        while pend:
            unit_back(*pend.pop(0))

        def v_chunks(cs):
            # V projection: token-major via X-stationary matmuls, both kv
            for c in cs:
                vp = psum.tile([128, NKV * HD], F32, tag="pacc", bufs=1,
                               name="vp")
                for d in range(DCH):
                    nc.tensor.matmul(
                        vp, lhsT=xt[:, d, c * 128:(c + 1) * 128],
                        rhs=wv[:, d, :], start=(d == 0), stop=(d == DCH - 1))
                nc.vector.tensor_copy(
                    vsb[:, :, c * 128:(c + 1) * 128],
                    vp.rearrange("p (k t) -> p k t", k=NKV))

        def q_unit_front(i, half):
            return unit_front(qnw, wq, slice(i * HD, (i + 1) * HD), half)

        # ---- attention (S^T orientation), software-pipelined j loop ------
        # emit denom/PV of block j-1 after scores of block j so the PE queue
        # never stalls on the current block's exp (ACT).
        def attn_qt(h, iqt):
            kv = h // 2
            ps_o = psum.tile([128, QT], F32, tag="pacc", bufs=1, name="ps_o")
            ps_d = psum.tile([128, QT], F32, tag="pd", bufs=1, name="ps_d")
            jmax = 8 * iqt + 8

            def scores(j):
                c0 = max(0, 128 * j - QT * iqt)
                c0a = (c0 // 512) * 512   # bank-aligned start for denom/PV
                ks = slice(128 * j, 128 * j + 128)
                ps_s = psum.tile([128, QT], F32, tag="pp", bufs=2, name="ps_s")
                diag = j >= 8 * iqt
                # compute scores + exp only from the exact causal start c0
                # (128-aligned, still within one bank per chunk)
                for n0, ne in _chunks512(c0, QT):
                    first = diag and n0 == c0
                    nc.tensor.matmul(
                        ps_s[:, n0:ne], lhsT=khm[kv][:, ks],
                        rhs=qhm[h][:, iqt * QT + n0: iqt * QT + ne],
                        start=True, stop=not first)
                    if first:
                        # triangular -inf on the diagonal block [c0, c0+128)
                        nc.tensor.matmul(
                            ps_s[:, c0:c0 + 128], lhsT=ident,
                            rhs=tri, start=False, stop=True)
                pt = pts.tile([128, QT], BF16, tag="pt", name="pt")
                if c0 > c0a:
                    # denom/PV read bank-aligned [c0a:); zero the pad region
                    nc.gpsimd.memset(pt[:, c0a:c0], 0.0)
                nc.scalar.activation(out=pt[:, c0:QT], in_=ps_s[:, c0:QT],
                                     func=AF.Exp)
                return j, c0a, pt

            def denoms(st):
                j, c0a, pt = st
                for n0, ne in _chunks512(c0a, QT):
                    jl = min(jmax - 1, 8 * iqt + n0 // 128 + 3)
                    nc.tensor.matmul(ps_d[:, n0:ne], lhsT=ones,
                                     rhs=pt[:, n0:ne],
                                     start=(j == 0), stop=(j == jl))

            def pvs(st):
                j, c0a, pt = st
                kvs = slice(128 * j, 128 * j + 128)
                for n0, ne in _chunks512(c0a, QT):
                    jl = min(jmax - 1, 8 * iqt + n0 // 128 + 3)
                    nc.tensor.matmul(ps_o[:, n0:ne],
                                     lhsT=vsb[:, kv, kvs],
                                     rhs=pt[:, n0:ne],
                                     start=(j == 0), stop=(j == jl))

            # pair-pipelined: scores for blocks (j, j+1), then the previous
            # pair's denominators back-to-back (ones lhsT dedupes) and PVs
            prev = None
            for jp in range(0, jmax, 2):
                st0 = scores(jp)
                st1 = scores(jp + 1)
                if prev is not None:
                    denoms(prev[0]); denoms(prev[1])
                    pvs(prev[0]); pvs(prev[1])
                prev = (st0, st1)
            denoms(prev[0]); denoms(prev[1])
            pvs(prev[0]); pvs(prev[1])
            rb = trans.tile([128, QT], F32, tag="rb", name="rb")
            nc.vector.reciprocal(out=rb, in_=ps_d)
            nc.vector.tensor_mul(
                attn[h][:, iqt * QT:(iqt + 1) * QT], ps_o, rb)

        def o_proj(c):
            po = psum.tile([128, D], F32, tag=("pp" if c % 2 == 0 else "pd"),
                           bufs=2 if c % 2 == 0 else 1, name="po")
            ts = slice(c * 128, (c + 1) * 128)
            for hh in range(NQ):
                for n0 in range(0, D, 512):
                    nc.tensor.matmul(po[:, n0:n0 + 512],
                                     lhsT=attn[hh][:, ts],
                                     rhs=wo[:, hh, n0:n0 + 512],
                                     start=(hh == 0), stop=(hh == NQ - 1))
            ob = pts.tile([128, D], F32, tag="ob", bufs=2, name="ob")
            nc.vector.tensor_copy(ob, po)
            nc.sync.dma_start(out=out_d[c], in_=ob)

        # Interleave: while attention for plane h runs (PE+ACT heavy), emit
        # the next q plane's projection front (PE) and, after, its norm/rope
        # back stage (DVE heavy).  o_proj for the first half overlaps the
        # second half's attention; its output DMA overlaps everything.
        if parts != "planes":
            v_chunks(range(NCH))
            for h in range(NQ):
                nxt = []
                if h + 1 < NQ:
                    nxt = [q_unit_front(h + 1, 0), q_unit_front(h + 1, 1)]
                attn_qt(h, 0)
                if h == NQ - 1 and parts != "noproj":
                    for c in range(8):
                        o_proj(c)
                if nxt:
                    unit_back(qhm[h + 1], nxt[0])
                attn_qt(h, 1)
                if nxt:
                    unit_back(qhm[h + 1], nxt[1])
            if parts != "noproj":
                for c in range(8, NCH):
                    o_proj(c)

    return nc


def legalize_waits(bir_bytes):
    """This walrus build rejects compute instructions with more than one
    sync wait.  Hoist all but one wait of each instruction into standalone
    EventSemaphore (pure wait) instructions on the same engine queue, which
    is semantically identical (in-order engine queues)."""
    import json
    m = json.loads(bir_bytes)
    n_fix = 0
    for f in m["functions"]:
        for blk in f["blocks"]:
            # drop Ldweights identical to the previously-kept one (the
            # stationary operand is still loaded; bass re-emits per matmul).
            # Safe: Ldweights carry no on_update; waits (rare) are kept.
            out0 = []
            last_key = None
            for ins in blk["instructions"]:
                if ins["opcode"] == "Ldweights":
                    si = ins.get("sync_info") or {}
                    key = json.dumps(
                        [ins.get("ins"), ins.get("outs"),
                         ins.get("perf_mode"), ins.get("tile_position")])
                    if (key == last_key and not si.get("on_wait")
                            and not si.get("on_update")):
                        continue
                    last_key = key
                out0.append(ins)
            blk["instructions"] = out0
            out = []
            for ins in blk["instructions"]:
                si = ins.get("sync_info")
                waits = (si or {}).get("on_wait") or []
                if len(waits) > 1 and ins["opcode"] != "EventSemaphore":
                    for i, w in enumerate(waits[:-1]):
                        out.append({
                            "debug": ins.get("debug", 0),
                            "engine": ins["engine"],
                            "ins": [], "outs": [],
                            "name": f"{ins['name']}-hw{i}",
                            "opcode": "EventSemaphore",
                            "sync_info": {"on_update": [], "on_wait": [w]},
                        })
                    si["on_wait"] = [waits[-1]]
                    n_fix += 1
                out.append(ins)
            blk["instructions"] = out
    return json.dumps(m).encode()


def _wrap_pi(x):
    return np.mod(x + np.pi, 2 * np.pi) - np.pi


def _prep_core_inputs(c, hidden_states, position_ids, q_w, k_w, v_w, o_w,
                      q_norm_w, k_norm_w):
    b, g = divmod(c, TPG)
    bf = ml_dtypes.bfloat16
    xt = np.ascontiguousarray(
        np.asarray(hidden_states[b], np.float32).T).astype(bf).reshape(DCH, 128, T)
    wq = np.ascontiguousarray(
        np.asarray(q_w[NQ * HD * g: NQ * HD * (g + 1)], np.float32).T
    ).astype(bf).reshape(DCH, 128, NQ * HD)
    wk = np.ascontiguousarray(
        np.asarray(k_w[NKV * HD * g: NKV * HD * (g + 1)], np.float32).T
    ).astype(bf).reshape(DCH, 128, NKV * HD)
    wv = np.ascontiguousarray(
        np.asarray(v_w[NKV * HD * g: NKV * HD * (g + 1)], np.float32).T
    ).astype(bf).reshape(DCH, 128, NKV * HD)
    wo = np.ascontiguousarray(
        np.asarray(o_w[:, NQ * HD * g: NQ * HD * (g + 1)], np.float32).T
    ).astype(bf).reshape(NQ, 128, D)

    pos = np.asarray(position_ids[b], np.float64)                      # [T]
    inv = 1.0 / (ROPE_BASE ** (np.arange(0, HD, 2, dtype=np.float64) / HD))
    invf2 = np.concatenate([inv, inv])                                 # [128]
    invf2s = np.concatenate([-inv, inv])
    ph = pos[None, :] * invf2[:, None]
    phs = pos[None, :] * invf2s[:, None]
    fqc = _wrap_pi(ph + np.pi / 2).astype(np.float32)      # sin(x)=cos(phase)
    fqs = _wrap_pi(phs).astype(np.float32)                 # signed sin
    qnw = (np.asarray(q_norm_w, np.float32) / math.sqrt(HD)).reshape(128, 1)
    knw = np.asarray(k_norm_w, np.float32).reshape(128, 1)

    ones = np.ones((128, 128), bf)
    ident = np.eye(128, dtype=np.float32).astype(bf)
    tri = np.where(np.arange(128)[:, None] <= np.arange(128)[None, :],
                   0.0, MASK_NEG).astype(np.float32).astype(bf)
    perm = np.zeros((128, 128), np.float32)
    perm[(np.arange(128) + 64) % 128, np.arange(128)] = 1.0
    perm = perm.astype(bf)
    return dict(xt=xt, wq=wq, wk=wk, wv=wv, wo=wo, fqc=fqc, fqs=fqs,
                qnw=qnw, knw=knw, ones=ones, ident=ident, tri=tri,
                perm=perm)


def kernel(hidden_states, position_ids, q_w, k_w, v_w, o_w, q_norm_w,
           k_norm_w):
    from concourse.bass_utils import run_bass_kernel_spmd

    nc = build_nc()
    orig_ser = nc.to_json_bytes
    nc.to_json_bytes = lambda: legalize_waits(orig_ser())
    in_maps = [
        _prep_core_inputs(c, hidden_states, position_ids, q_w, k_w, v_w, o_w,
                          q_norm_w, k_norm_w)
        for c in range(NCORES)
    ]
    res = run_bass_kernel_spmd(nc, in_maps, list(range(NCORES))).results
    out = np.zeros((B, L, D), np.float32)
    for c in range(NCORES):
        out[c // TPG] += np.asarray(res[c]["out"], np.float32).reshape(L, D)
    return out



# revision 2
# speedup vs baseline: 225.2587x; 225.2587x over previous
"""Bass/Trainium2 kernel for nn_Attention_19481971654841.

Full attention block: q/k/v proj + per-head RMSNorm(q,k) + RoPE + causal GQA
SDPA + o_proj.  B=2, L=2048, D=1024, H=16, KVH=8, HD=128.

Sharding (8 NeuronCores): data-parallel over batch (2 groups of 4 cores) x
4-way tensor-parallel over heads inside each group.  Core c handles batch
c//4 and q-heads [4g:4g+4), kv-heads [2g:2g+2) with g = c%4.  Each core
produces a partial [L, D] o_proj contribution; host sums the 4 partials per
batch.

Per-core dataflow (all matmuls bf16, fp32 PSUM accumulate):
  - projections produce Q^T/K^T head-major [HD=128 part, T] (weights
    stationary on PE); V token-major [T, HD] via X-stationary matmuls.
  - RMSNorm in head-major: sum-of-squares via all-ones matmul (partition
    reduce, broadcast to 128 rows), then r = exp(-0.5*ln(ms+eps)) via two
    ACT passes — Ln and Exp live in the same ACT table set
    (natural_log_exp_and_others), so no table reloads against the
    attention exp; fused (q * w) * r via scalar_tensor_tensor.
  - RoPE with host-prepared bf16 cos/sin tables; the sin table carries the
    rotate-half sign per partition; rotate-half itself is a partition-swap
    SBUF->SBUF DMA (no PE, no PSUM).
  - attention in S^T orientation: scores[kv, q] = K_hm^T . Q_hm, causal mask
    folded in as an identity-matmul bias add, ACT Exp -> P^T bf16, softmax
    denominator via all-ones matmul (broadcast rows) accumulating from the
    exact causal start, PV accumulates attn^T[hd, q] directly; 1/denom =
    exp(-ln(denom)) on ACT (keeps DVE off the critical path).
  - o_proj: attn^T slices are the stationary lhsT; partial out written fp32
    from PSUM straight to DRAM.
"""

import math
from contextlib import ExitStack

import numpy as np
import ml_dtypes

import concourse.bass as bass
import concourse.mybir as mybir
import concourse.tile as tile

F32 = mybir.dt.float32
BF16 = mybir.dt.bfloat16
AF = mybir.ActivationFunctionType
ALU = mybir.AluOpType

# problem constants (per spec; hardcoded — kernel.py must be self-contained)
B, L, D = 2, 2048, 1024
H, KVH, HD = 16, 8, 128
EPS = 1e-6
ROPE_BASE = 1000000.0

# per-core constants
NCORES = 8
TPG = 4                 # tensor-parallel group size (cores per batch)
T = L                   # tokens per core (one batch element)
NQ = H // TPG           # 4 q heads per core
NKV = KVH // TPG        # 2 kv heads per core
DCH = D // 128          # 8 input-dim chunks
QT = 1024               # q tile width (PSUM tile [128, 1024] f32 = 2 banks)
NQT = T // QT           # 2 q tiles
NCH = T // 128          # 16 token chunks
MASK_NEG = -30000.0


def _chunks512(c0, end):
    """matmul col chunks covering [c0, end), each within one PSUM bank"""
    out = []
    n0 = c0
    while n0 < end:
        ne = min(end, (n0 // 512 + 1) * 512)
        out.append((n0, ne))
        n0 = ne
    return out


def build_nc(parts="full"):
    nc = bass.Bass()

    xt_d = nc.dram_tensor("xt", [DCH, 128, T], BF16, kind="ExternalInput")
    wq_d = nc.dram_tensor("wq", [128, DCH, NQ * HD], BF16, kind="ExternalInput")
    wk_d = nc.dram_tensor("wk", [128, DCH, NKV * HD], BF16, kind="ExternalInput")
    wv_d = nc.dram_tensor("wv", [128, DCH, NKV * HD], BF16, kind="ExternalInput")
    wo_d = nc.dram_tensor("wo", [128, NQ, D], BF16, kind="ExternalInput")
    ctab_d = nc.dram_tensor("ctab", [128, T], BF16, kind="ExternalInput")
    stab_d = nc.dram_tensor("stab", [128, T], BF16, kind="ExternalInput")
    qnw_d = nc.dram_tensor("qnw", [128, 1], F32, kind="ExternalInput")
    knw_d = nc.dram_tensor("knw", [128, 1], F32, kind="ExternalInput")
    ones_d = nc.dram_tensor("ones", [128, 128], BF16, kind="ExternalInput")
    ident_d = nc.dram_tensor("ident", [128, 128], BF16, kind="ExternalInput")
    tri_d = nc.dram_tensor("tri", [128, 128], BF16, kind="ExternalInput")
    out_d = nc.dram_tensor("out", [NCH, 128, D], F32, kind="ExternalOutput")

    with tile.TileContext(nc) as tc, ExitStack() as ctx:
        sing = ctx.enter_context(tc.tile_pool(name="sing", bufs=1))
        trans = ctx.enter_context(tc.tile_pool(name="trans", bufs=2))
        pts = ctx.enter_context(tc.tile_pool(name="pts", bufs=4))
        psum = ctx.enter_context(tc.tile_pool(name="psum", bufs=1, space="PSUM"))

        # ---- persistent loads (wk/wq first, xt chunked: the first proj
        # matmuls only need wk + xt chunk 0, so PE starts early) ------------
        wq = sing.tile([128, DCH, NQ * HD], BF16, tag="wq")
        wk = sing.tile([128, DCH, NKV * HD], BF16, tag="wk")
        wv = sing.tile([128, DCH, NKV * HD], BF16, tag="wv")
        nc.sync.dma_start(out=wk, in_=wk_d[:, :, :])
        xt = sing.tile([128, DCH, T], BF16, tag="xt")
        for d in range(DCH):
            nc.sync.dma_start(out=xt[:, d, :], in_=xt_d[d])
        nc.sync.dma_start(out=wq, in_=wq_d[:, :, :])
        nc.sync.dma_start(out=wv, in_=wv_d[:, :, :])
        wo = sing.tile([128, NQ, D], BF16, tag="wo")
        nc.sync.dma_start(out=wo, in_=wo_d[:, :, :])
        ctab = sing.tile([128, T], BF16, tag="ctab")
        stab = sing.tile([128, T], BF16, tag="stab")
        nc.sync.dma_start(out=ctab, in_=ctab_d[:, :])
        nc.sync.dma_start(out=stab, in_=stab_d[:, :])
        qnw = sing.tile([128, 1], F32, tag="qnw")
        knw = sing.tile([128, 1], F32, tag="knw")
        nc.sync.dma_start(out=qnw, in_=qnw_d[:, :])
        nc.sync.dma_start(out=knw, in_=knw_d[:, :])
        ones = sing.tile([128, 128], BF16, tag="ones")
        ident = sing.tile([128, 128], BF16, tag="ident")
        tri = sing.tile([128, 128], BF16, tag="tri")
        nc.sync.dma_start(out=ones, in_=ones_d[:, :])
        nc.sync.dma_start(out=ident, in_=ident_d[:, :])
        nc.sync.dma_start(out=tri, in_=tri_d[:, :])
        epsb = sing.tile([128, 1], F32, tag="epsb")
        nc.vector.memset(epsb, EPS)

        # ---- persistent plane outputs -----------------------------------
        khm = [sing.tile([128, T], BF16, tag=f"khm{i}", name=f"khm{i}") for i in range(NKV)]
        qhm = [sing.tile([128, T], BF16, tag=f"qhm{i}", name=f"qhm{i}") for i in range(NQ)]
        vsb = sing.tile([128, NKV, T], BF16, tag="vsb")
        attn = [sing.tile([128, T], BF16, tag=f"attn{i}", name=f"attn{i}") for i in range(NQ)]

        # ---- norm + rope pipeline, split into stages so plane units can
        # be software-pipelined against each other and against attention.
        # Unit = (plane, half).  Stage F (front): projection matmuls + early
        # PSUM drain to bf16 + square.  Stage B (back): partition-reduce
        # sum-of-squares (ones matmul), r = exp(-0.5 ln(ms+eps)) on ACT,
        # fused (q * w) * r, rotate-half via partition-swap DMA, rope muls.
        def unit_front(w_ap, wmat, fslice, half):
            qp = psum.tile([128, QT], F32, tag="pp", bufs=2, name="qp")
            for d in range(DCH):
                for n0 in range(0, QT, 512):
                    nc.tensor.matmul(
                        qp[:, n0:n0 + 512],
                        lhsT=wmat[:, d, fslice],
                        rhs=xt[:, d, half * QT + n0: half * QT + n0 + 512],
                        start=(d == 0), stop=(d == DCH - 1),
                    )
            qc = trans.tile([128, QT], BF16, tag="qc", bufs=3, name="qc")
            nc.vector.tensor_copy(qc, qp)      # early PSUM drain
            sq = trans.tile([128, QT], BF16, tag="sq", bufs=2, name="sq")
            nc.vector.tensor_mul(sq, qc, qc)
            return (w_ap, half, qc, sq)

        def unit_back(plane_out, st):
            w_ap, half, qc, sq = st
            ssq = psum.tile([128, QT], F32, tag="pacc", bufs=1, name="ssq")
            for n0 in range(0, QT, 512):
                nc.tensor.matmul(ssq[:, n0:n0 + 512], lhsT=ones,
                                 rhs=sq[:, n0:n0 + 512], start=True, stop=True)
            # r = 1/sqrt(ms+eps) = exp(-0.5*ln(ms+eps)); Ln+Exp share one
            # ACT table set so this never evicts the attention Exp tables
            lnv = trans.tile([128, QT], F32, tag="ss", name="lnv")
            nc.scalar.activation(out=lnv, in_=ssq, func=AF.Ln,
                                 scale=1.0 / HD, bias=epsb)
            rr = trans.tile([128, QT], F32, tag="rr", name="rr")
            nc.scalar.activation(out=rr, in_=lnv, func=AF.Exp, scale=-0.5)
            qn = trans.tile([128, QT], BF16, tag="qn", name="qn")
            nc.vector.scalar_tensor_tensor(
                out=qn, in0=qc, scalar=w_ap, in1=rr,
                op0=ALU.mult, op1=ALU.mult)
            # rope: rotate-half = partition swap, done by SBUF->SBUF DMA;
            # sign is folded into the sin table (stab rows 0:64 hold -sin)
            cs = slice(half * QT, half * QT + QT)
            mc = trans.tile([128, QT], BF16, tag="mc", name="mc")
            nc.vector.tensor_mul(mc, qn, ctab[:, cs])
            rot = trans.tile([128, QT], BF16, tag="rot", name="rot")
            nc.sync.dma_start(out=rot[0:64, :], in_=qn[64:128, :])
            nc.sync.dma_start(out=rot[64:128, :], in_=qn[0:64, :])
            msw = trans.tile([128, QT], BF16, tag="msw", name="msw")
            nc.vector.tensor_mul(msw, rot, stab[:, cs])
            # final rope add on the (otherwise idle) gpsimd engine
            nc.gpsimd.tensor_add(plane_out[:, cs], mc, msw)

        # kv plane units, depth-2 pipelined
        kunits = [(khm[i], knw, wk, slice(i * HD, (i + 1) * HD), half)
                  for i in range(NKV) for half in range(NQT)]
        kunits += [(qhm[0], qnw, wq, slice(0, HD), half)
                   for half in range(NQT)]
        pend = []
        for plane, w_ap, wmat, fsl, half in kunits:
            st = unit_front(w_ap, wmat, fsl, half)
            pend.append((plane, st))
            if len(pend) > 1:
                unit_back(*pend.pop(0))
        while pend:
            unit_back(*pend.pop(0))

        def v_chunks(cs):
            # V projection: token-major via X-stationary matmuls, both kv
            for c in cs:
                vp = psum.tile([128, NKV * HD], F32, tag="pacc", bufs=1,
                               name="vp")
                for d in range(DCH):
                    nc.tensor.matmul(
                        vp, lhsT=xt[:, d, c * 128:(c + 1) * 128],
                        rhs=wv[:, d, :], start=(d == 0), stop=(d == DCH - 1))
                nc.vector.tensor_copy(
                    vsb[:, :, c * 128:(c + 1) * 128],
                    vp.rearrange("p (k t) -> p k t", k=NKV))

        def q_unit_front(i, half):
            return unit_front(qnw, wq, slice(i * HD, (i + 1) * HD), half)

        # ---- attention (S^T orientation), software-pipelined j loop ------
        # emit denom/PV of block j-1 after scores of block j so the PE queue
        # never stalls on the current block's exp (ACT).
        def attn_qt(h, iqt):
            kv = h // 2
            ps_o = psum.tile([128, QT], F32, tag="pacc", bufs=1, name="ps_o")
            ps_d = psum.tile([128, QT], F32, tag="pd", bufs=1, name="ps_d")
            jmax = 8 * iqt + 8

            def scores(j):
                c0 = max(0, 128 * j - QT * iqt)
                ks = slice(128 * j, 128 * j + 128)
                ps_s = psum.tile([128, QT], F32, tag="pp", bufs=2, name="ps_s")
                diag = j >= 8 * iqt
                # compute scores + exp only from the exact causal start c0
                # (128-aligned, still within one bank per chunk)
                for n0, ne in _chunks512(c0, QT):
                    first = diag and n0 == c0
                    nc.tensor.matmul(
                        ps_s[:, n0:ne], lhsT=khm[kv][:, ks],
                        rhs=qhm[h][:, iqt * QT + n0: iqt * QT + ne],
                        start=True, stop=not first)
                    if first:
                        # triangular -inf on the diagonal block [c0, c0+128)
                        nc.tensor.matmul(
                            ps_s[:, c0:c0 + 128], lhsT=ident,
                            rhs=tri, start=False, stop=True)
                pt = pts.tile([128, QT], BF16, tag="pt", name="pt")
                nc.scalar.activation(out=pt[:, c0:QT], in_=ps_s[:, c0:QT],
                                     func=AF.Exp)
                return j, c0, pt

            def denoms(st):
                j, c0, pt = st
                for n0, ne in _chunks512(c0, QT):
                    jl = min(jmax - 1, 8 * iqt + (ne - 1) // 128)
                    nc.tensor.matmul(ps_d[:, n0:ne], lhsT=ones,
                                     rhs=pt[:, n0:ne],
                                     start=(j == 0), stop=(j == jl))

            def pvs(st):
                j, c0, pt = st
                kvs = slice(128 * j, 128 * j + 128)
                for n0, ne in _chunks512(c0, QT):
                    jl = min(jmax - 1, 8 * iqt + (ne - 1) // 128)
                    nc.tensor.matmul(ps_o[:, n0:ne],
                                     lhsT=vsb[:, kv, kvs],
                                     rhs=pt[:, n0:ne],
                                     start=(j == 0), stop=(j == jl))

            # pair-pipelined: scores for blocks (j, j+1), then the previous
            # pair's denominators back-to-back (ones lhsT dedupes) and PVs
            prev = None
            for jp in range(0, jmax, 2):
                st0 = scores(jp)
                st1 = scores(jp + 1)
                if prev is not None:
                    denoms(prev[0]); denoms(prev[1])
                    pvs(prev[0]); pvs(prev[1])
                prev = (st0, st1)
            denoms(prev[0]); denoms(prev[1])
            pvs(prev[0]); pvs(prev[1])
            # 1/denom = exp(-ln(denom)) on ACT (same table set as Exp)
            lnd = trans.tile([128, QT], F32, tag="rb", name="lnd")
            nc.scalar.activation(out=lnd, in_=ps_d, func=AF.Ln)
            rb = trans.tile([128, QT], F32, tag="rb2", name="rb")
            nc.scalar.activation(out=rb, in_=lnd, func=AF.Exp, scale=-1.0)
            nc.vector.tensor_mul(
                attn[h][:, iqt * QT:(iqt + 1) * QT], ps_o, rb)

        def o_proj(c):
            po = psum.tile([128, D], F32, tag=("pp" if c % 2 == 0 else "pd"),
                           bufs=2 if c % 2 == 0 else 1, name="po")
            ts = slice(c * 128, (c + 1) * 128)
            for hh in range(NQ):
                for n0 in range(0, D, 512):
                    nc.tensor.matmul(po[:, n0:n0 + 512],
                                     lhsT=attn[hh][:, ts],
                                     rhs=wo[:, hh, n0:n0 + 512],
                                     start=(hh == 0), stop=(hh == NQ - 1))
            ob = pts.tile([128, D], F32, tag="ob", bufs=2, name="ob")
            nc.vector.tensor_copy(ob, po)
            nc.sync.dma_start(out=out_d[c], in_=ob)

        # Interleave: while attention for plane h runs (PE+ACT heavy), emit
        # the next q plane's projection front (PE) and, after, its norm/rope
        # back stage (DVE heavy).  o_proj for the first half overlaps the
        # second half's attention; its output DMA overlaps everything.
        if parts != "planes":
            v_chunks(range(NCH))
            for h in range(NQ):
                nxt = []
                if h + 1 < NQ:
                    nxt = [q_unit_front(h + 1, 0), q_unit_front(h + 1, 1)]
                attn_qt(h, 0)
                if h == NQ - 1 and parts != "noproj":
                    for c in range(8):
                        o_proj(c)
                if nxt:
                    unit_back(qhm[h + 1], nxt[0])
                attn_qt(h, 1)
                if nxt:
                    unit_back(qhm[h + 1], nxt[1])
            if parts != "noproj":
                for c in range(8, NCH):
                    o_proj(c)

    return nc


def legalize_waits(bir_bytes):
    """This walrus build rejects compute instructions with more than one
    sync wait.  Hoist all but one wait of each instruction into standalone
    EventSemaphore (pure wait) instructions on the same engine queue, which
    is semantically identical (in-order engine queues)."""
    import json
    m = json.loads(bir_bytes)
    n_fix = 0
    for f in m["functions"]:
        for blk in f["blocks"]:
            # drop Ldweights identical to the previously-kept one (the
            # stationary operand is still loaded; bass re-emits per matmul).
            # Safe: Ldweights carry no on_update; waits (rare) are kept.
            out0 = []
            last_key = None
            for ins in blk["instructions"]:
                if ins["opcode"] == "Ldweights":
                    si = ins.get("sync_info") or {}
                    key = json.dumps(
                        [ins.get("ins"), ins.get("outs"),
                         ins.get("perf_mode"), ins.get("tile_position")])
                    if (key == last_key and not si.get("on_wait")
                            and not si.get("on_update")):
                        continue
                    last_key = key
                out0.append(ins)
            blk["instructions"] = out0
            out = []
            for ins in blk["instructions"]:
                si = ins.get("sync_info")
                waits = (si or {}).get("on_wait") or []
                if len(waits) > 1 and ins["opcode"] != "EventSemaphore":
                    for i, w in enumerate(waits[:-1]):
                        out.append({
                            "debug": ins.get("debug", 0),
                            "engine": ins["engine"],
                            "ins": [], "outs": [],
                            "name": f"{ins['name']}-hw{i}",
                            "opcode": "EventSemaphore",
                            "sync_info": {"on_update": [], "on_wait": [w]},
                        })
                    si["on_wait"] = [waits[-1]]
                    n_fix += 1
                out.append(ins)
            blk["instructions"] = out
    return json.dumps(m).encode()


def _prep_core_inputs(c, hidden_states, position_ids, q_w, k_w, v_w, o_w,
                      q_norm_w, k_norm_w):
    b, g = divmod(c, TPG)
    bf = ml_dtypes.bfloat16
    xt = np.ascontiguousarray(
        np.asarray(hidden_states[b], np.float32).T).astype(bf).reshape(DCH, 128, T)
    # weights host-transposed to [128 partitions, DCH, f] so each DMA line
    # is one long contiguous run per partition (big packets)
    wq = np.ascontiguousarray(
        np.asarray(q_w[NQ * HD * g: NQ * HD * (g + 1)], np.float32).T
        .reshape(DCH, 128, NQ * HD).transpose(1, 0, 2)).astype(bf)
    wk = np.ascontiguousarray(
        np.asarray(k_w[NKV * HD * g: NKV * HD * (g + 1)], np.float32).T
        .reshape(DCH, 128, NKV * HD).transpose(1, 0, 2)).astype(bf)
    wv = np.ascontiguousarray(
        np.asarray(v_w[NKV * HD * g: NKV * HD * (g + 1)], np.float32).T
        .reshape(DCH, 128, NKV * HD).transpose(1, 0, 2)).astype(bf)
    wo = np.ascontiguousarray(
        np.asarray(o_w[:, NQ * HD * g: NQ * HD * (g + 1)], np.float32).T
        .reshape(NQ, 128, D).transpose(1, 0, 2)).astype(bf)

    pos = np.asarray(position_ids[b], np.float64)                      # [T]
    inv = 1.0 / (ROPE_BASE ** (np.arange(0, HD, 2, dtype=np.float64) / HD))
    invf2 = np.concatenate([inv, inv])                                 # [128]
    invf2s = np.concatenate([-inv, inv])
    ctab = np.cos(pos[None, :] * invf2[:, None]).astype(np.float32).astype(bf)
    stab = np.sin(pos[None, :] * invf2s[:, None]).astype(np.float32).astype(bf)
    qnw = (np.asarray(q_norm_w, np.float32) / math.sqrt(HD)).reshape(128, 1)
    knw = np.asarray(k_norm_w, np.float32).reshape(128, 1)

    ones = np.ones((128, 128), bf)
    ident = np.eye(128, dtype=np.float32).astype(bf)
    tri = np.where(np.arange(128)[:, None] <= np.arange(128)[None, :],
                   0.0, MASK_NEG).astype(np.float32).astype(bf)
    return dict(xt=xt, wq=wq, wk=wk, wv=wv, wo=wo, ctab=ctab, stab=stab,
                qnw=qnw, knw=knw, ones=ones, ident=ident, tri=tri)


def kernel(hidden_states, position_ids, q_w, k_w, v_w, o_w, q_norm_w,
           k_norm_w):
    from concourse.bass_utils import run_bass_kernel_spmd

    nc = build_nc()
    orig_ser = nc.to_json_bytes
    nc.to_json_bytes = lambda: legalize_waits(orig_ser())
    in_maps = [
        _prep_core_inputs(c, hidden_states, position_ids, q_w, k_w, v_w, o_w,
                          q_norm_w, k_norm_w)
        for c in range(NCORES)
    ]
    res = run_bass_kernel_spmd(nc, in_maps, list(range(NCORES))).results
    out = np.zeros((B, L, D), np.float32)
    for c in range(NCORES):
        out[c // TPG] += np.asarray(res[c]["out"], np.float32).reshape(L, D)
    return out


# revision 13
# speedup vs baseline: 265.2098x; 1.1774x over previous
"""Bass/Trainium2 kernel for nn_Attention_19481971654841.

Full attention block: q/k/v proj + per-head RMSNorm(q,k) + RoPE + causal GQA
SDPA + o_proj.  B=2, L=2048, D=1024, H=16, KVH=8, HD=128.

Sharding (8 NeuronCores): data-parallel over batch (2 groups of 4 cores) x
4-way tensor-parallel over heads inside each group.  Core c handles batch
c//4 and q-heads [4g:4g+4), kv-heads [2g:2g+2) with g = c%4.  Each core
produces a partial [L, D] o_proj contribution; host sums the 4 partials per
batch.

Per-core dataflow (all matmuls bf16, fp32 PSUM accumulate):
  - projections produce Q^T/K^T head-major [HD=128 part, T] (weights
    stationary on PE); V token-major [T, HD] via X-stationary matmuls.
  - RMSNorm in head-major: sum-of-squares via all-ones matmul (partition
    reduce, broadcast to 128 rows), then r = exp(-0.5*ln(ms+eps)) via two
    ACT passes — Ln and Exp live in the same ACT table set
    (natural_log_exp_and_others), so no table reloads against the
    attention exp; fused (q * w) * r via scalar_tensor_tensor.
  - RoPE with host-prepared bf16 cos/sin tables; the sin table carries the
    rotate-half sign per partition; rotate-half itself is a partition-swap
    SBUF->SBUF DMA (no PE, no PSUM).
  - attention in S^T orientation: scores[kv, q] = K_hm^T . Q_hm, causal mask
    folded in as an identity-matmul bias add, ACT Exp -> P^T bf16, softmax
    denominator via all-ones matmul (broadcast rows) accumulating from the
    exact causal start, PV accumulates attn^T[hd, q] directly; 1/denom =
    exp(-ln(denom)) on ACT (keeps DVE off the critical path).
  - o_proj: attn^T slices are the stationary lhsT; partial out written fp32
    from PSUM straight to DRAM.
"""

import math
from contextlib import ExitStack

import numpy as np
import ml_dtypes

import concourse.bass as bass
import concourse.mybir as mybir
import concourse.tile as tile

F32 = mybir.dt.float32
BF16 = mybir.dt.bfloat16
AF = mybir.ActivationFunctionType
ALU = mybir.AluOpType

# problem constants (per spec; hardcoded — kernel.py must be self-contained)
B, L, D = 2, 2048, 1024
H, KVH, HD = 16, 8, 128
EPS = 1e-6
ROPE_BASE = 1000000.0

# per-core constants
NCORES = 8
TPG = 4                 # tensor-parallel group size (cores per batch)
T = L                   # tokens per core (one batch element)
NQ = H // TPG           # 4 q heads per core
NKV = KVH // TPG        # 2 kv heads per core
DCH = D // 128          # 8 input-dim chunks
QT = 1024               # q tile width (PSUM tile [128, 1024] f32 = 2 banks)
NQT = T // QT           # 2 q tiles
NCH = T // 128          # 16 token chunks
MASK_NEG = -30000.0


def _chunks512(c0, end):
    """matmul col chunks covering [c0, end), each within one PSUM bank"""
    out = []
    n0 = c0
    while n0 < end:
        ne = min(end, (n0 // 512 + 1) * 512)
        out.append((n0, ne))
        n0 = ne
    return out


def build_nc(parts="full"):
    nc = bass.Bass()

    xt_d = nc.dram_tensor("xt", [DCH, 128, T], BF16, kind="ExternalInput")
    wq_d = nc.dram_tensor("wq", [128, DCH, NQ * HD], BF16, kind="ExternalInput")
    wk_d = nc.dram_tensor("wk", [128, DCH, NKV * HD], BF16, kind="ExternalInput")
    wv_d = nc.dram_tensor("wv", [128, DCH, NKV * HD], BF16, kind="ExternalInput")
    wo_d = nc.dram_tensor("wo", [128, NQ, D], BF16, kind="ExternalInput")
    ctab_d = nc.dram_tensor("ctab", [128, T], BF16, kind="ExternalInput")
    stab_d = nc.dram_tensor("stab", [128, T], BF16, kind="ExternalInput")
    qnw_d = nc.dram_tensor("qnw", [128, 1], F32, kind="ExternalInput")
    knw_d = nc.dram_tensor("knw", [128, 1], F32, kind="ExternalInput")
    ones_d = nc.dram_tensor("ones", [128, 128], BF16, kind="ExternalInput")
    ident_d = nc.dram_tensor("ident", [128, 128], BF16, kind="ExternalInput")
    tri_d = nc.dram_tensor("tri", [128, 128], BF16, kind="ExternalInput")
    out_d = nc.dram_tensor("out", [NCH, 128, D], F32, kind="ExternalOutput")

    with tile.TileContext(nc) as tc, ExitStack() as ctx:
        sing = ctx.enter_context(tc.tile_pool(name="sing", bufs=1))
        trans = ctx.enter_context(tc.tile_pool(name="trans", bufs=2))
        pts = ctx.enter_context(tc.tile_pool(name="pts", bufs=4))
        psum = ctx.enter_context(tc.tile_pool(name="psum", bufs=1, space="PSUM"))

        # ---- persistent loads (wk/wq first, xt chunked: the first proj
        # matmuls only need wk + xt chunk 0, so PE starts early) ------------
        wq = sing.tile([128, DCH, NQ * HD], BF16, tag="wq")
        wk = sing.tile([128, DCH, NKV * HD], BF16, tag="wk")
        wv = sing.tile([128, DCH, NKV * HD], BF16, tag="wv")
        nc.sync.dma_start(out=wk, in_=wk_d[:, :, :])
        xt = sing.tile([128, DCH, T], BF16, tag="xt")
        for d in range(DCH):
            nc.sync.dma_start(out=xt[:, d, :], in_=xt_d[d])
        nc.sync.dma_start(out=wq, in_=wq_d[:, :, :])
        nc.sync.dma_start(out=wv, in_=wv_d[:, :, :])
        wo = sing.tile([128, NQ, D], BF16, tag="wo")
        nc.sync.dma_start(out=wo, in_=wo_d[:, :, :])
        ctab = sing.tile([128, T], BF16, tag="ctab")
        stab = sing.tile([128, T], BF16, tag="stab")
        nc.sync.dma_start(out=ctab, in_=ctab_d[:, :])
        nc.sync.dma_start(out=stab, in_=stab_d[:, :])
        qnw = sing.tile([128, 1], F32, tag="qnw")
        knw = sing.tile([128, 1], F32, tag="knw")
        nc.sync.dma_start(out=qnw, in_=qnw_d[:, :])
        nc.sync.dma_start(out=knw, in_=knw_d[:, :])
        ones = sing.tile([128, 128], BF16, tag="ones")
        ident = sing.tile([128, 128], BF16, tag="ident")
        tri = sing.tile([128, 128], BF16, tag="tri")
        nc.sync.dma_start(out=ones, in_=ones_d[:, :])
        nc.sync.dma_start(out=ident, in_=ident_d[:, :])
        nc.sync.dma_start(out=tri, in_=tri_d[:, :])
        epsb = sing.tile([128, 1], F32, tag="epsb")
        nc.vector.memset(epsb, EPS)

        # ---- persistent plane outputs -----------------------------------
        khm = [sing.tile([128, T], BF16, tag=f"khm{i}", name=f"khm{i}") for i in range(NKV)]
        qhm = [sing.tile([128, T], BF16, tag=f"qhm{i}", name=f"qhm{i}") for i in range(NQ)]
        vsb = sing.tile([128, NKV, T], BF16, tag="vsb")
        attn = [sing.tile([128, T], BF16, tag=f"attn{i}", name=f"attn{i}") for i in range(NQ)]

        # ---- norm + rope pipeline, split into stages so plane units can
        # be software-pipelined against each other and against attention.
        # Unit = (plane, half).  Stage F (front): projection matmuls + early
        # PSUM drain to bf16 + square.  Stage B (back): partition-reduce
        # sum-of-squares (ones matmul), r = exp(-0.5 ln(ms+eps)) on ACT,
        # fused (q * w) * r, rotate-half via partition-swap DMA, rope muls.
        def unit_front(w_ap, wmat, fslice, half):
            qp = psum.tile([128, QT], F32, tag="pp", bufs=2, name="qp")
            for d in range(DCH):
                for n0 in range(0, QT, 512):
                    nc.tensor.matmul(
                        qp[:, n0:n0 + 512],
                        lhsT=wmat[:, d, fslice],
                        rhs=xt[:, d, half * QT + n0: half * QT + n0 + 512],
                        start=(d == 0), stop=(d == DCH - 1),
                    )
            qc = trans.tile([128, QT], BF16, tag="qc", bufs=3, name="qc")
            nc.vector.tensor_copy(qc, qp)      # early PSUM drain
            sq = trans.tile([128, QT], BF16, tag="sq", bufs=2, name="sq")
            nc.vector.tensor_mul(sq, qc, qc)
            return (w_ap, half, qc, sq)

        def unit_back(plane_out, st):
            w_ap, half, qc, sq = st
            ssq = psum.tile([128, QT], F32, tag="pacc", bufs=1, name="ssq")
            for n0 in range(0, QT, 512):
                nc.tensor.matmul(ssq[:, n0:n0 + 512], lhsT=ones,
                                 rhs=sq[:, n0:n0 + 512], start=True, stop=True)
            # r = 1/sqrt(ms+eps) = exp(-0.5*ln(ms+eps)); Ln+Exp share one
            # ACT table set so this never evicts the attention Exp tables
            lnv = trans.tile([128, QT], F32, tag="ss", name="lnv")
            nc.scalar.activation(out=lnv, in_=ssq, func=AF.Ln,
                                 scale=1.0 / HD, bias=epsb)
            rr = trans.tile([128, QT], BF16, tag="rr", name="rr")
            nc.scalar.activation(out=rr, in_=lnv, func=AF.Exp, scale=-0.5)
            qn = trans.tile([128, QT], BF16, tag="qn", name="qn")
            nc.vector.scalar_tensor_tensor(
                out=qn, in0=qc, scalar=w_ap, in1=rr,
                op0=ALU.mult, op1=ALU.mult)
            # rope: rotate-half = partition swap, done by SBUF->SBUF DMA;
            # sign is folded into the sin table (stab rows 0:64 hold -sin)
            cs = slice(half * QT, half * QT + QT)
            mc = trans.tile([128, QT], BF16, tag="mc", name="mc")
            nc.vector.tensor_mul(mc, qn, ctab[:, cs])
            rot = trans.tile([128, QT], BF16, tag="rot", name="rot")
            nc.sync.dma_start(out=rot[0:64, :], in_=qn[64:128, :])
            nc.sync.dma_start(out=rot[64:128, :], in_=qn[0:64, :])
            msw = trans.tile([128, QT], BF16, tag="msw", name="msw")
            nc.vector.tensor_mul(msw, rot, stab[:, cs])
            nc.vector.tensor_add(plane_out[:, cs], mc, msw)

        def kv_pipeline(half):
            # k0, k1, q0 planes for one half, depth-2 front/back pipeline
            units = [(khm[i], knw, wk, slice(i * HD, (i + 1) * HD))
                     for i in range(NKV)]
            units += [(qhm[0], qnw, wq, slice(0, HD))]
            pend = []
            for plane, w_ap, wmat, fsl in units:
                st = unit_front(w_ap, wmat, fsl, half)
                pend.append((plane, st))
                if len(pend) > 1:
                    unit_back(*pend.pop(0))
            while pend:
                unit_back(*pend.pop(0))

        def v_chunks(cs):
            # V projection: token-major via X-stationary matmuls, both kv
            for c in cs:
                vp = psum.tile([128, NKV * HD], F32, tag="pacc", bufs=1,
                               name="vp")
                for d in range(DCH):
                    nc.tensor.matmul(
                        vp, lhsT=xt[:, d, c * 128:(c + 1) * 128],
                        rhs=wv[:, d, :], start=(d == 0), stop=(d == DCH - 1))
                nc.vector.tensor_copy(
                    vsb[:, :, c * 128:(c + 1) * 128],
                    vp.rearrange("p (k t) -> p k t", k=NKV))

        def q_unit_front(i, half):
            return unit_front(qnw, wq, slice(i * HD, (i + 1) * HD), half)

        # ---- attention (S^T orientation), software-pipelined j loop ------
        # emit denom/PV of block j-1 after scores of block j so the PE queue
        # never stalls on the current block's exp (ACT).
        def attn_qt(h, iqt):
            kv = h // 2
            ps_o = psum.tile([128, QT], F32, tag="pacc", bufs=1, name="ps_o")
            ps_d = psum.tile([128, QT], F32, tag="pd", bufs=1, name="ps_d")
            jmax = 8 * iqt + 8

            def scores(j):
                c0 = max(0, 128 * j - QT * iqt)
                ks = slice(128 * j, 128 * j + 128)
                ps_s = psum.tile([128, QT], F32, tag="pp", bufs=2, name="ps_s")
                diag = j >= 8 * iqt
                # compute scores + exp only from the exact causal start c0
                # (128-aligned, still within one bank per chunk)
                for n0, ne in _chunks512(c0, QT):
                    first = diag and n0 == c0
                    nc.tensor.matmul(
                        ps_s[:, n0:ne], lhsT=khm[kv][:, ks],
                        rhs=qhm[h][:, iqt * QT + n0: iqt * QT + ne],
                        start=True, stop=not first)
                    if first:
                        # triangular -inf on the diagonal block [c0, c0+128)
                        nc.tensor.matmul(
                            ps_s[:, c0:c0 + 128], lhsT=ident,
                            rhs=tri, start=False, stop=True)
                pt = pts.tile([128, QT], BF16, tag="pt", name="pt")
                nc.scalar.activation(out=pt[:, c0:QT], in_=ps_s[:, c0:QT],
                                     func=AF.Exp)
                return j, c0, pt

            def denoms(st):
                # 4x column-tiled partial sums: group g = j%4 accumulates
                # into psum partitions [32g, 32g+32) concurrently with the
                # other groups (distinct PE col-groups, own XBUS each), so
                # the denominator reduce streams at ~4 cols/cycle instead
                # of 1.  j == 0 initializes all four groups full-width.
                j, c0, pt = st
                o32 = ones[:, 0:32]
                if j == 0:
                    # row+col-tiled init: group g starts from the sum of its
                    # own 32-row kv strip of block 0, so the four partials
                    # add up to exactly the block-0 contribution
                    for g in range(4):
                        for n0, ne in _chunks512(0, QT):
                            nc.tensor.matmul(
                                ps_d[32 * g:32 * g + 32, n0:ne],
                                lhsT=ones[32 * g:32 * g + 32, 0:32],
                                rhs=pt[32 * g:32 * g + 32, n0:ne], start=True,
                                stop=(jmax - 4 + g == 0),
                                tile_position=(32 * g, 32 * g),
                                skip_group_check=True)
                else:
                    g = j % 4
                    for n0, ne in _chunks512(c0, QT):
                        nc.tensor.matmul(
                            ps_d[32 * g:32 * g + 32, n0:ne], lhsT=o32,
                            rhs=pt[:, n0:ne], start=False,
                            stop=(j == jmax - 4 + g),
                            tile_position=(0, 32 * g),
                            skip_group_check=True)

            def pvs(st):
                j, c0, pt = st
                kvs = slice(128 * j, 128 * j + 128)
                for n0, ne in _chunks512(c0, QT):
                    jl = min(jmax - 1, 8 * iqt + (ne - 1) // 128)
                    nc.tensor.matmul(ps_o[:, n0:ne],
                                     lhsT=vsb[:, kv, kvs],
                                     rhs=pt[:, n0:ne],
                                     start=(j == 0), stop=(j == jl),
                                     skip_group_check=True)

            # pair-pipelined: scores for blocks (j, j+1), then the previous
            # pair's denominators back-to-back (ones lhsT dedupes) and PVs
            prev = None
            for jp in range(0, jmax, 2):
                st0 = scores(jp)
                st1 = scores(jp + 1)
                if prev is not None:
                    denoms(prev[0]); denoms(prev[1])
                    pvs(prev[0]); pvs(prev[1])
                prev = (st0, st1)
            denoms(prev[0]); denoms(prev[1])
            pvs(prev[0]); pvs(prev[1])
            # combine the 4 partial-sum groups + broadcast to 128 rows with
            # one full ones-matmul (reads the bf16 SBUF drain of ps_d)
            pd_sb = pts.tile([128, QT], BF16, tag="pt", name="pd_sb")
            nc.vector.tensor_copy(pd_sb, ps_d)
            for n0 in range(0, QT, 512):
                nc.tensor.matmul(ps_d[:, n0:n0 + 512], lhsT=ones,
                                 rhs=pd_sb[:, n0:n0 + 512],
                                 start=True, stop=True,
                                 skip_group_check=True)
            # 1/denom = exp(-ln(denom)) on ACT (same table set as Exp);
            # the combine summed 32 broadcast copies of each of the 4
            # partials, so rescale by 1/32 inside the Ln
            lnd = trans.tile([128, QT], F32, tag="rb", name="lnd")
            nc.scalar.activation(out=lnd, in_=ps_d, func=AF.Ln, scale=1.0 / 32)
            rb = trans.tile([128, QT], F32, tag="rb2", name="rb")
            nc.scalar.activation(out=rb, in_=lnd, func=AF.Exp, scale=-1.0)
            nc.vector.tensor_mul(
                attn[h][:, iqt * QT:(iqt + 1) * QT], ps_o, rb)

        def o_proj(c):
            po = psum.tile([128, D], F32, tag=("pp" if c % 2 == 0 else "pd"),
                           bufs=2 if c % 2 == 0 else 1, name="po")
            ts = slice(c * 128, (c + 1) * 128)
            for hh in range(NQ):
                for n0 in range(0, D, 512):
                    nc.tensor.matmul(po[:, n0:n0 + 512],
                                     lhsT=attn[hh][:, ts],
                                     rhs=wo[:, hh, n0:n0 + 512],
                                     start=(hh == 0), stop=(hh == NQ - 1))
            ob = pts.tile([128, D], F32, tag="ob", bufs=2, name="ob")
            nc.vector.tensor_copy(ob, po)
            nc.sync.dma_start(out=out_d[c], in_=ob)

        # Schedule: half-0 planes + first-half V first so attn(0,0) starts
        # as early as possible; half-1 planes + rest of V overlap attn(0,0).
        # Thereafter each q plane's projection front runs one attention tile
        # ahead of its norm/rope back stage, which runs one tile ahead of
        # its use (so the back's ACT/DVE chain hides under attention PE).
        # o_proj for the first token half overlaps the last head's second
        # half; its output DMA overlaps everything.
        if parts != "planes":
            kv_pipeline(0)
            v_chunks(range(8))
            attn_qt(0, 0)
            kv_pipeline(1)
            v_chunks(range(8, NCH))
            nxt = [q_unit_front(1, 0), q_unit_front(1, 1)]
            unit_back(qhm[1], nxt[0])
            attn_qt(0, 1)
            unit_back(qhm[1], nxt[1])
            for h in range(1, NQ):
                nxt = []
                if h + 1 < NQ:
                    nxt = [q_unit_front(h + 1, 0), q_unit_front(h + 1, 1)]
                attn_qt(h, 0)
                if h == NQ - 1 and parts != "noproj":
                    for c in range(8):
                        o_proj(c)
                if nxt:
                    unit_back(qhm[h + 1], nxt[0])
                attn_qt(h, 1)
                if nxt:
                    unit_back(qhm[h + 1], nxt[1])
            if parts != "noproj":
                for c in range(8, NCH):
                    o_proj(c)

    return nc


def legalize_waits(bir_bytes):
    """This walrus build rejects compute instructions with more than one
    sync wait.  Hoist all but one wait of each instruction into standalone
    EventSemaphore (pure wait) instructions on the same engine queue, which
    is semantically identical (in-order engine queues)."""
    import json
    m = json.loads(bir_bytes)
    n_fix = 0
    for f in m["functions"]:
        for blk in f["blocks"]:
            # drop Ldweights identical to the previously-kept one (the
            # stationary operand is still loaded; bass re-emits per matmul).
            # Safe: Ldweights carry no on_update; waits (rare) are kept.
            out0 = []
            last_key = None
            for ins in blk["instructions"]:
                if ins["opcode"] == "Ldweights":
                    si = ins.get("sync_info") or {}
                    key = json.dumps(
                        [ins.get("ins"), ins.get("outs"),
                         ins.get("perf_mode"), ins.get("tile_position")])
                    if (key == last_key and not si.get("on_wait")
                            and not si.get("on_update")):
                        continue
                    last_key = key
                out0.append(ins)
            blk["instructions"] = out0
            out = []
            for ins in blk["instructions"]:
                si = ins.get("sync_info")
                waits = (si or {}).get("on_wait") or []
                if len(waits) > 1 and ins["opcode"] != "EventSemaphore":
                    for i, w in enumerate(waits[:-1]):
                        out.append({
                            "debug": ins.get("debug", 0),
                            "engine": ins["engine"],
                            "ins": [], "outs": [],
                            "name": f"{ins['name']}-hw{i}",
                            "opcode": "EventSemaphore",
                            "sync_info": {"on_update": [], "on_wait": [w]},
                        })
                    si["on_wait"] = [waits[-1]]
                    n_fix += 1
                out.append(ins)
            blk["instructions"] = out
    return json.dumps(m).encode()


def _prep_core_inputs(c, hidden_states, position_ids, q_w, k_w, v_w, o_w,
                      q_norm_w, k_norm_w):
    b, g = divmod(c, TPG)
    bf = ml_dtypes.bfloat16
    xt = np.ascontiguousarray(
        np.asarray(hidden_states[b], np.float32).T).astype(bf).reshape(DCH, 128, T)
    # weights host-transposed to [128 partitions, DCH, f] so each DMA line
    # is one long contiguous run per partition (big packets)
    wq = np.ascontiguousarray(
        np.asarray(q_w[NQ * HD * g: NQ * HD * (g + 1)], np.float32).T
        .reshape(DCH, 128, NQ * HD).transpose(1, 0, 2)).astype(bf)
    wk = np.ascontiguousarray(
        np.asarray(k_w[NKV * HD * g: NKV * HD * (g + 1)], np.float32).T
        .reshape(DCH, 128, NKV * HD).transpose(1, 0, 2)).astype(bf)
    wv = np.ascontiguousarray(
        np.asarray(v_w[NKV * HD * g: NKV * HD * (g + 1)], np.float32).T
        .reshape(DCH, 128, NKV * HD).transpose(1, 0, 2)).astype(bf)
    wo = np.ascontiguousarray(
        np.asarray(o_w[:, NQ * HD * g: NQ * HD * (g + 1)], np.float32).T
        .reshape(NQ, 128, D).transpose(1, 0, 2)).astype(bf)

    pos = np.asarray(position_ids[b], np.float64)                      # [T]
    inv = 1.0 / (ROPE_BASE ** (np.arange(0, HD, 2, dtype=np.float64) / HD))
    invf2 = np.concatenate([inv, inv])                                 # [128]
    invf2s = np.concatenate([-inv, inv])
    ctab = np.cos(pos[None, :] * invf2[:, None]).astype(np.float32).astype(bf)
    stab = np.sin(pos[None, :] * invf2s[:, None]).astype(np.float32).astype(bf)
    qnw = (np.asarray(q_norm_w, np.float32) / math.sqrt(HD)).reshape(128, 1)
    knw = np.asarray(k_norm_w, np.float32).reshape(128, 1)

    ones = np.ones((128, 128), bf)
    ident = np.eye(128, dtype=np.float32).astype(bf)
    tri = np.where(np.arange(128)[:, None] <= np.arange(128)[None, :],
                   0.0, MASK_NEG).astype(np.float32).astype(bf)
    return dict(xt=xt, wq=wq, wk=wk, wv=wv, wo=wo, ctab=ctab, stab=stab,
                qnw=qnw, knw=knw, ones=ones, ident=ident, tri=tri)


def kernel(hidden_states, position_ids, q_w, k_w, v_w, o_w, q_norm_w,
           k_norm_w):
    from concourse.bass_utils import run_bass_kernel_spmd

    nc = build_nc()
    orig_ser = nc.to_json_bytes
    nc.to_json_bytes = lambda: legalize_waits(orig_ser())
    in_maps = [
        _prep_core_inputs(c, hidden_states, position_ids, q_w, k_w, v_w, o_w,
                          q_norm_w, k_norm_w)
        for c in range(NCORES)
    ]
    res = run_bass_kernel_spmd(nc, in_maps, list(range(NCORES))).results
    out = np.zeros((B, L, D), np.float32)
    for c in range(NCORES):
        out[c // TPG] += np.asarray(res[c]["out"], np.float32).reshape(L, D)
    return out
